# revision 24
# baseline (speedup 1.0000x reference)
"""Trainium2 Bass kernel for nn_EnvironmentEmbedder.

Sharding: pure data parallel. Core i processes batch slice [128*i : 128*(i+1)],
with batch elements mapped to SBUF partitions ([128, free] tiles everywhere).

The kernel is HBM-bandwidth-bound (per-core traffic at f32 everywhere is
140.8 MB ~= the ~375 GB/s/core roofline), so I/O dtypes are chosen to cut
traffic while staying inside the harness rel-err gate:
  - embedded_static/dynamic stay f32: their sum cancels, so input rounding
    would turn into unbounded *relative* output error near zeros.
  - every other input is >= 0 with no cancellation anywhere downstream, so
    bf16 input rounding stays ~0.4% relative -> shipped as a bf16 pack.
  - the output is written bf16 (rounding of an exact f32 result is always
    relative, <= 0.4%) and upcast to f32 on the host.
Per-core traffic: 81.92 MB env f32 in + 3.68 MB pack bf16 + 25.76 MB out bf16
= 111.4 MB -> ~297 us DMA floor. Measured end-to-end max rel err: 1.08e-2.

Scheduling lessons baked in (each measured on HW):
  - The load stream is WAR-coupled to the env compute chain through the
    s/d tile slots. Keep that chain SHORT (DVE add ~2.7 us + mul ~3.2 us
    per 4-channel chunk) and the slot ring DEEP (bufs=4 -> 34 us of slack)
    so DVE jitter never starves the DMA queues. A GpSimd add in this chain
    (53 G elem/s, 13 us/chunk) paced the whole stream: 423 us.
  - GpSimd gets only latency-tolerant work in ops it verifiably supports
    (tensor_add/tensor_mul/tensor_copy f32): obs-premultiplies and the
    channel-sums as pairwise add trees. TensorScalarPtr (scalar_tensor_
    tensor / AP-scalar ops) does NOT lower on Pool.
  - Output stores issue from the Scalar engine's HWDGE ring (loads use the
    SP ring), deferred one iteration, so a store waiting on compute can
    never head-of-line-block loads. A single-ring variant lost 70 us to
    exactly that.
  - DVE ops writing bf16 cost ~20-50% more than their f32 forms; CAST is
    cheap (~234 G elem/s). Stage slots are still written directly in bf16
    (one pass) — only the chsum takes an f32 scratch + tiny cast because
    its pairwise tree would otherwise round intermediates.

Per-core output layout ([128, 161*625] bf16, channel-major):
  ch   0..127  (static_c + dynamic_c) * obs      streamed, add+mul on DVE
  ch 128       obstacle * obs
  ch 129       observability_current * obs
  ch 130       obs * obs
  ch 131..136  shuffle(prev_visitations)_j * 0.5 * obs
  ch 137       sum_k(vis_k) * obs
  ch 138       leader * obs
  ch 139       follower * obs
  ch 140..145  shuffle(all_prev_targets)_j * 0.5 * obs
  ch 146..151  shuffle(previous_target)_j * obs
  ch 152       0.5 * sum_k(atgt_k) * obs
  ch 153       sum_k(ptgt_k) * obs
  ch 154       1.0
  ch 155..160  one_hot(rot)
where obs := observability_in_memory.

The egocentric shuffle out_j = x_{(j - rot) % 6} is computed with per-partition
one-hot masks R_r = (rot == r):  out_j = sum_r R_r * x_{(j-r)%6} (exactly one
mask is nonzero per partition, so accumulating in a bf16 slot stays exact).
The obs multiply is folded in by premultiplying the 6 source channels by obs
once, and the 0.5 scaling is folded into the masks.
"""

import sys

sys.path.insert(0, "/opt/trn_rl_repo")

from contextlib import ExitStack

import numpy as np
import ml_dtypes

import concourse.bass as bass
import concourse.tile as tile
from concourse import bacc, mybir
from concourse.bass_utils import run_bass_kernel_spmd

F32 = mybir.dt.float32
BF16 = mybir.dt.bfloat16
ALU = mybir.AluOpType
NPBF = ml_dtypes.bfloat16

B = 1024
N_CORES = 8
BS = B // N_CORES  # 128 batch elements per core = SBUF partitions
EMB = 128
HW = 625  # 25*25
NROT = 6
NCH = EMB + 33  # 161 output channels
NOBS = 4  # obs copies kept resident (2500 wide = one env chunk)
GP_ADD_CHUNKS = range(4, 21)  # env chunks whose add runs on GpSimd

ENV_CHUNK = 4  # env channels per streamed input tile (f32: 10 KB DMA rows)
# bf16 pack layout per partition: obs, 4 singles, 3x 6-ch tensors, rot-as-f
PACK_LAYOUT = [("obs", HW), ("obstacle", HW), ("ocur", HW), ("leader", HW),
               ("follower", HW), ("vis", NROT * HW), ("atgt", NROT * HW),
               ("ptgt", NROT * HW), ("rot", 1)]
PACK_W = sum(w for _, w in PACK_LAYOUT)  # 14376 bf16 per partition
STAGE_CHUNKS = [(128, 6), (134, 6), (140, 6), (146, 6), (152, 6),
                (158, 3)]  # (start_ch, n_ch)


def build_body(nc, tc, ctx, t_in, t_out):
    pool = ctx.enter_context(tc.tile_pool(name="resident", bufs=1))
    stage_pool = ctx.enter_context(tc.tile_pool(name="stage", bufs=2))
    env_s_pool = ctx.enter_context(tc.tile_pool(name="env_s", bufs=4))
    env_d_pool = ctx.enter_context(tc.tile_pool(name="env_d", bufs=4))
    env_o_pool = ctx.enter_context(tc.tile_pool(name="env_o", bufs=3))

    # ---- resident load: all small tensors host-packed into one bf16 DMA ----
    pack_t = pool.tile([BS, PACK_W], BF16, tag="pack")
    nc.sync.dma_start(pack_t[:], t_in["small_pack"][:])
    obs_b = pack_t[:, 0:HW]
    sing_b = pack_t[:, HW:5 * HW]
    vapt_b = pack_t[:, 5 * HW:(5 + 3 * NROT) * HW]
    rot_b = pack_t[:, PACK_W - 1:PACK_W]

    # ---- upconvert to f32 working set ----
    obs_rep = pool.tile([BS, NOBS * HW], F32, tag="obs_rep")
    nc.vector.tensor_copy(obs_rep[:, 0:HW], obs_b)
    nc.vector.tensor_copy(obs_rep[:, HW:2 * HW], obs_rep[:, 0:HW])
    nc.vector.tensor_copy(obs_rep[:, 2 * HW:4 * HW], obs_rep[:, 0:2 * HW])
    obs_t = obs_rep[:, 0:HW]

    sing_f = pool.tile([BS, 4 * HW], F32, tag="sing")
    nc.vector.tensor_copy(sing_f[:], sing_b)
    obst_t = sing_f[:, 0:HW]
    ocur_t = sing_f[:, HW:2 * HW]
    lead_t = sing_f[:, 2 * HW:3 * HW]
    foll_t = sing_f[:, 3 * HW:4 * HW]

    # vis/atgt/ptgt upconverted (DVE cast), then premultiplied by obs on
    # GpSimd (latency-tolerant). vis first: earliest consumers.
    prem = pool.tile([BS, 3 * NROT * HW], F32, tag="prem")
    nc.vector.tensor_copy(prem[:], vapt_b)
    vis_t = prem[:, 0:NROT * HW]
    atgt_t = prem[:, NROT * HW:2 * NROT * HW]
    ptgt_t = prem[:, 2 * NROT * HW:3 * NROT * HW]
    for k in range(3):
        base = k * NROT * HW
        nc.gpsimd.tensor_mul(prem[:, base:base + NOBS * HW],
                             prem[:, base:base + NOBS * HW], obs_rep[:])
        nc.gpsimd.tensor_mul(
            prem[:, base + NOBS * HW:base + NROT * HW],
            prem[:, base + NOBS * HW:base + NROT * HW],
            obs_rep[:, 0:(NROT - NOBS) * HW])

    rotf = pool.tile([BS, 1], F32, tag="rotf")
    nc.vector.tensor_copy(rotf[:], rot_b)

    # f32 scratch for the chsum accumulation (bf16 intermediates would
    # compound rounding past the error budget)
    csum = pool.tile([BS, HW], F32, tag="csum")

    # ---- per-partition one-hot rotation masks ----
    R = []   # R[r]  = (rot == r)            [128, 1] f32
    Rh = []  # Rh[r] = 0.5 * (rot == r)
    for r in range(NROT):
        rt = pool.tile([BS, 1], F32, tag=f"R{r}")
        nc.vector.tensor_scalar(rt[:], rotf[:], float(r), None,
                                op0=ALU.is_equal)
        R.append(rt)
        rh = pool.tile([BS, 1], F32, tag=f"Rh{r}")
        nc.vector.tensor_scalar_mul(rh[:], rt[:], 0.5)
        Rh.append(rh)

    def emit_shuffle(slot, xp, masks, j):
        # slot = sum_r masks[r] * xp[:, ((j - r) % 6)]
        nc.scalar.mul(slot, xp[:, j * HW:(j + 1) * HW], masks[0][:])
        for r in range(1, NROT):
            k = (j - r) % NROT
            nc.vector.scalar_tensor_tensor(
                slot, xp[:, k * HW:(k + 1) * HW], masks[r][:], slot,
                op0=ALU.mult, op1=ALU.add)

    def emit_chsum(slot, xp):
        # sum of 6 non-negative channels as sequential GpSimd f32 adds;
        # only the final DVE copy rounds to bf16.
        nc.gpsimd.tensor_add(csum[:], xp[:, 0:HW], xp[:, HW:2 * HW])
        for k in range(2, NROT):
            nc.gpsimd.tensor_add(csum[:], csum[:], xp[:, k * HW:(k + 1) * HW])
        nc.vector.tensor_copy(slot, csum[:])

    def emit_channel(ch, slot):
        if ch == 128:
            nc.vector.tensor_mul(slot, obst_t, obs_t)
        elif ch == 129:
            nc.vector.tensor_mul(slot, ocur_t, obs_t)
        elif ch == 130:
            nc.vector.tensor_mul(slot, obs_t, obs_t)
        elif 131 <= ch <= 136:
            emit_shuffle(slot, vis_t, Rh, ch - 131)
        elif ch == 137:
            emit_chsum(slot, vis_t)
        elif ch == 138:
            nc.vector.tensor_mul(slot, lead_t, obs_t)
        elif ch == 139:
            nc.vector.tensor_mul(slot, foll_t, obs_t)
        elif 140 <= ch <= 145:
            emit_shuffle(slot, atgt_t, Rh, ch - 140)
        elif 146 <= ch <= 151:
            emit_shuffle(slot, ptgt_t, R, ch - 146)
        elif ch == 152:
            emit_chsum(slot, atgt_t)
            nc.vector.tensor_scalar_mul(slot, slot, 0.5)
        elif ch == 153:
            emit_chsum(slot, ptgt_t)
        elif ch == 154:
            nc.vector.memset(slot, 1.0)
        else:  # 155..160: compass one-hot = Identity(0*obs + R[r])
            nc.scalar.activation(
                slot, obs_t, mybir.ActivationFunctionType.Identity,
                bias=R[ch - 155][:], scale=0.0)

    # ---- env stream interleaved with the small channels ----
    # Per 4-channel chunk the DMA moves ~3.2 MB (~8.5 us); the DVE env chain
    # is ~6 us. Small channels average ~2.5 us/chunk of DVE; short-term
    # overruns are absorbed by the 4-deep s/d slot ring.
    ch_queue = []
    for ck, (start_ch, n_ch) in enumerate(STAGE_CHUNKS):
        for i in range(n_ch):
            ch_queue.append((ck, start_ch, n_ch, i))
    stage_tiles = {}

    pending_writes = []  # deferred (out_cols, tile) DMA stores

    def emit_small(budget):
        while budget > 0 and ch_queue:
            ck, start_ch, n_ch, i = ch_queue.pop(0)
            if ck not in stage_tiles:
                stage_tiles[ck] = stage_pool.tile(
                    [BS, n_ch * HW], BF16, tag="stage", name=f"stage{ck}")
            emit_channel(start_ch + i, stage_tiles[ck][:, i * HW:(i + 1) * HW])
            if i == n_ch - 1:
                pending_writes.append(
                    (slice(start_ch * HW, (start_ch + n_ch) * HW),
                     stage_tiles[ck]))
            budget -= 1

    w = ENV_CHUNK * HW
    env_total = EMB // ENV_CHUNK
    for c in range(env_total):
        cols = slice(c * w, (c + 1) * w)
        s_tile = env_s_pool.tile([BS, w], F32, tag="env_s")
        nc.sync.dma_start(s_tile[:], t_in["embedded_static"][:, cols])
        d_tile = env_d_pool.tile([BS, w], F32, tag="env_d")
        nc.sync.dma_start(d_tile[:], t_in["embedded_dynamic"][:, cols])
        for out_cols, tile_ in pending_writes:
            nc.scalar.dma_start(t_out[:, out_cols], tile_[:])
        pending_writes.clear()
        # Mid-stream adds run on the otherwise-idle GpSimd (the 4-deep slot
        # ring absorbs its higher latency); early/late ones stay on DVE,
        # which has slack there, keeping GpSimd clear of the premultiplies.
        if c in GP_ADD_CHUNKS:
            nc.gpsimd.tensor_add(s_tile[:], s_tile[:], d_tile[:])
        else:
            nc.vector.tensor_add(s_tile[:], s_tile[:], d_tile[:])
        o_tile = env_o_pool.tile([BS, w], BF16, tag="env_o")
        nc.vector.tensor_mul(o_tile[:], s_tile[:], obs_rep[:])
        pending_writes.append((cols, o_tile))
        if c >= 2:
            emit_small(2)
    emit_small(len(ch_queue))
    for out_cols, tile_ in pending_writes:
        nc.scalar.dma_start(t_out[:, out_cols], tile_[:])
    pending_writes.clear()


def build_nc():
    nc = bacc.Bacc("TRN2", target_bir_lowering=False, debug=False)
    t_in = {
        "embedded_static": nc.dram_tensor(
            "embedded_static", [BS, EMB * HW], F32, kind="ExternalInput"),
        "embedded_dynamic": nc.dram_tensor(
            "embedded_dynamic", [BS, EMB * HW], F32, kind="ExternalInput"),
        "small_pack": nc.dram_tensor(
            "small_pack", [BS, PACK_W], BF16, kind="ExternalInput"),
    }
    t_out = nc.dram_tensor("out", [BS, NCH * HW], BF16, kind="ExternalOutput")
    with tile.TileContext(nc) as tc, ExitStack() as ctx:
        build_body(nc, tc, ctx, t_in, t_out)
    nc.compile()
    return nc


def make_in_maps(inputs):
    arrs = {k: np.asarray(v) for k, v in inputs.items()}
    src = {
        "obs": arrs["observability_in_memory"].reshape(B, HW),
        "obstacle": arrs["obstacle_mask"].reshape(B, HW),
        "ocur": arrs["observability_current"].reshape(B, HW),
        "leader": arrs["leader_location"].reshape(B, HW),
        "follower": arrs["follower_location"].reshape(B, HW),
        "vis": arrs["previous_visitations"].reshape(B, NROT * HW),
        "atgt": arrs["all_previous_targets"].reshape(B, NROT * HW),
        "ptgt": arrs["previous_target"].reshape(B, NROT * HW),
        # rotations 0..5 are exactly representable as bf16 float values
        "rot": arrs["rotations"].reshape(B, 1).astype(np.float32),
    }
    flat = {
        "embedded_static": np.ascontiguousarray(
            arrs["embedded_static"].reshape(B, EMB * HW)),
        "embedded_dynamic": np.ascontiguousarray(
            arrs["embedded_dynamic"].reshape(B, EMB * HW)),
        "small_pack": np.concatenate(
            [src[name] for name, _ in PACK_LAYOUT], axis=1).astype(NPBF),
    }
    return [
        {k: v[i * BS:(i + 1) * BS] for k, v in flat.items()}
        for i in range(N_CORES)
    ]


def kernel(**inputs) -> np.ndarray:
    nc = build_nc()
    in_maps = make_in_maps(inputs)
    res = run_bass_kernel_spmd(nc, in_maps, list(range(N_CORES)))
    return np.concatenate(
        [r["out"].astype(np.float32).reshape(BS, NCH, 25, 25)
         for r in res.results], axis=0)


if __name__ == "__main__":
    rng = np.random.default_rng(0)
    demo = {
        "embedded_static": rng.standard_normal((B, EMB, 25, 25), np.float32),
        "embedded_dynamic": rng.standard_normal((B, EMB, 25, 25), np.float32),
        "obstacle_mask": rng.random((B, 25, 25), dtype=np.float32),
        "observability_current": rng.random((B, 25, 25), dtype=np.float32),
        "observability_in_memory": rng.random((B, 25, 25), dtype=np.float32),
        "previous_visitations": rng.random((B, NROT, 25, 25), dtype=np.float32),
        "all_previous_targets": rng.random((B, NROT, 25, 25), dtype=np.float32),
        "previous_target": rng.random((B, NROT, 25, 25), dtype=np.float32),
        "leader_location": rng.random((B, 25, 25), dtype=np.float32),
        "follower_location": rng.random((B, 25, 25), dtype=np.float32),
        "rotations": rng.integers(0, NROT, (B,), dtype=np.int32),
    }
    out = kernel(**demo)
    print("out", out.shape, out.dtype)


# revision 25
# speedup vs baseline: 1.1867x; 1.1867x over previous
"""Trainium2 Bass kernel for nn_EnvironmentEmbedder.

Sharding: pure data parallel. Core i processes batch slice [128*i : 128*(i+1)],
with batch elements mapped to SBUF partitions ([128, free] tiles everywhere).

The kernel is HBM-bandwidth-bound (per-core traffic at f32 everywhere is
140.8 MB ~= the ~375 GB/s/core roofline), so I/O dtypes are chosen to cut
traffic while staying inside the harness rel-err gate:
  - embedded_static/dynamic stay f32: their sum cancels, so input rounding
    would turn into unbounded *relative* output error near zeros. The host
    interleaves them chunk-wise into one DRAM tensor so each 4-channel env
    chunk is a single 20 KB-row DMA.
  - every other input is >= 0 with no cancellation anywhere downstream, so
    bf16 input rounding stays ~0.4% relative -> shipped as a bf16 pack.
  - the output is written bf16 (rounding of an exact f32 result is always
    relative, <= 0.4%) and upcast to f32 on the host.
Per-core traffic: 81.92 MB env f32 in + 3.68 MB pack bf16 + 25.76 MB out bf16
= 111.4 MB -> ~297 us DMA floor. Measured end-to-end max rel err: 1.08e-2.

Scheduling lessons baked in (each measured on HW):
  - The load stream is WAR-coupled to the env compute chain through the
    env tile slots. Keep that chain SHORT and ALL ON DVE (add ~2.8 us +
    mul->bf16 ~3.0 us per chunk) and the slot ring DEEP (bufs=4 -> 34 us
    slack). Putting the add on GpSimd (53 G elem/s) paced the stream to
    423 us when done for all chunks and 388 us when done mid-stream only:
    GpSimd's latency in this chain always loses.
  - GpSimd gets only latency-tolerant work in ops it verifiably supports
    (tensor_add/tensor_mul f32): obs-premultiplies and the channel-sums.
    TensorScalarPtr (AP-scalar ops) does not lower on Pool.
  - Output stores issue from the Scalar engine's HWDGE ring (loads use the
    SP ring), deferred one iteration, so a store waiting on compute can
    never head-of-line-block loads. A single-ring variant lost 70 us to
    exactly that.
  - The egocentric shuffle runs as 5 rotation steps of 2 contiguous
    multiply-add pieces covering all 6 channels (plus a Scalar-engine
    masked-copy init), amortizing DVE instruction overhead vs the naive
    36 ops of 625.

Per-core output layout ([128, 161*625] bf16, channel-major):
  ch   0..127  (static_c + dynamic_c) * obs      streamed, add+mul on DVE
  ch 128       obstacle * obs
  ch 129       observability_current * obs
  ch 130       obs * obs
  ch 131..136  shuffle(prev_visitations)_j * 0.5 * obs
  ch 137       sum_k(vis_k) * obs
  ch 138       leader * obs
  ch 139       follower * obs
  ch 140..145  shuffle(all_prev_targets)_j * 0.5 * obs
  ch 146..151  shuffle(previous_target)_j * obs
  ch 152       0.5 * sum_k(atgt_k) * obs
  ch 153       sum_k(ptgt_k) * obs
  ch 154       1.0
  ch 155..160  one_hot(rot)
where obs := observability_in_memory.

The shuffle out_j = x_{(j - rot) % 6} uses per-partition one-hot masks
R_r = (rot == r): out = sum_r R_r * rot_r(x), where rot_r(x) is a channel
rotation = 2 contiguous free-dim slices. Exactly one mask is nonzero per
partition, so accumulating in the bf16 slot stays exact. The obs multiply
is folded in by premultiplying the 6 source channels by obs once, and the
0.5 scaling is folded into the masks.
"""

import sys

sys.path.insert(0, "/opt/trn_rl_repo")

from contextlib import ExitStack

import numpy as np
import ml_dtypes

import concourse.bass as bass
import concourse.tile as tile
from concourse import bacc, mybir
from concourse.bass_utils import run_bass_kernel_spmd

F32 = mybir.dt.float32
BF16 = mybir.dt.bfloat16
ALU = mybir.AluOpType
NPBF = ml_dtypes.bfloat16

B = 1024
N_CORES = 8
BS = B // N_CORES  # 128 batch elements per core = SBUF partitions
EMB = 128
HW = 625  # 25*25
NROT = 6
NCH = EMB + 33  # 161 output channels
NOBS = 4  # obs copies kept resident (2500 wide = one env chunk)

ENV_CHUNK = 4  # env channels per streamed tile (s+d interleaved: 20 KB rows)
# bf16 pack layout per partition: obs, 4 singles, 3x 6-ch tensors, rot-as-f
PACK_LAYOUT = [("obs", HW), ("obstacle", HW), ("ocur", HW), ("leader", HW),
               ("follower", HW), ("vis", NROT * HW), ("atgt", NROT * HW),
               ("ptgt", NROT * HW), ("rot", 1)]
PACK_W = sum(w for _, w in PACK_LAYOUT)  # 14376 bf16 per partition
# stage tiles group contiguous output channels; shuffle blocks need their 6
# channels in ONE tile so rotation pieces can span them
STAGE_CHUNKS = [(128, 3), (131, 6), (137, 3), (140, 6), (146, 6), (152, 3),
                (155, 6)]  # (start_ch, n_ch)


def build_body(nc, tc, ctx, t_in, t_out):
    pool = ctx.enter_context(tc.tile_pool(name="resident", bufs=1))
    stage_pool = ctx.enter_context(tc.tile_pool(name="stage", bufs=2))
    env_pool = ctx.enter_context(tc.tile_pool(name="env", bufs=4))
    env_o_pool = ctx.enter_context(tc.tile_pool(name="env_o", bufs=3))

    # ---- resident load: obs region first so the f32 working set (which
    # gates the first env mul) is ready ~10 us earlier ----
    pack_t = pool.tile([BS, PACK_W], BF16, tag="pack")
    nc.sync.dma_start(pack_t[:, 0:HW], t_in["small_pack"][:, 0:HW])
    nc.sync.dma_start(pack_t[:, HW:PACK_W], t_in["small_pack"][:, HW:PACK_W])
    obs_b = pack_t[:, 0:HW]
    sing_b = pack_t[:, HW:5 * HW]
    vapt_b = pack_t[:, 5 * HW:(5 + 3 * NROT) * HW]
    rot_b = pack_t[:, PACK_W - 1:PACK_W]

    # ---- upconvert to f32 working set ----
    obs_rep = pool.tile([BS, NOBS * HW], F32, tag="obs_rep")
    nc.vector.tensor_copy(obs_rep[:, 0:HW], obs_b)
    nc.vector.tensor_copy(obs_rep[:, HW:2 * HW], obs_rep[:, 0:HW])
    nc.vector.tensor_copy(obs_rep[:, 2 * HW:4 * HW], obs_rep[:, 0:2 * HW])
    obs_t = obs_rep[:, 0:HW]

    sing_f = pool.tile([BS, 4 * HW], F32, tag="sing")
    nc.vector.tensor_copy(sing_f[:], sing_b)

    # vis/atgt/ptgt upconverted (DVE cast), then premultiplied by obs on
    # GpSimd (latency-tolerant). vis first: earliest consumers.
    prem = pool.tile([BS, 3 * NROT * HW], F32, tag="prem")
    nc.vector.tensor_copy(prem[:], vapt_b)
    vis_t = prem[:, 0:NROT * HW]
    atgt_t = prem[:, NROT * HW:2 * NROT * HW]
    ptgt_t = prem[:, 2 * NROT * HW:3 * NROT * HW]
    for k in range(3):
        base = k * NROT * HW
        nc.gpsimd.tensor_mul(prem[:, base:base + NOBS * HW],
                             prem[:, base:base + NOBS * HW], obs_rep[:])
        nc.gpsimd.tensor_mul(
            prem[:, base + NOBS * HW:base + NROT * HW],
            prem[:, base + NOBS * HW:base + NROT * HW],
            obs_rep[:, 0:(NROT - NOBS) * HW])

    rotf = pool.tile([BS, 1], F32, tag="rotf")
    nc.vector.tensor_copy(rotf[:], rot_b)

    # f32 scratch for the chsum accumulation (bf16 intermediates would
    # compound rounding past the error budget)
    csum = pool.tile([BS, HW], F32, tag="csum")

    # ---- per-partition one-hot rotation masks ----
    R = []   # R[r]  = (rot == r)            [128, 1] f32
    Rh = []  # Rh[r] = 0.5 * (rot == r)
    for r in range(NROT):
        rt = pool.tile([BS, 1], F32, tag=f"R{r}")
        nc.vector.tensor_scalar(rt[:], rotf[:], float(r), None,
                                op0=ALU.is_equal)
        R.append(rt)
        rh = pool.tile([BS, 1], F32, tag=f"Rh{r}")
        nc.vector.tensor_scalar_mul(rh[:], rt[:], 0.5)
        Rh.append(rh)

    def shuffle_init(block, xp, masks):
        # r=0 (identity rotation): block = masks[0] * xp over all 6 channels,
        # on the Scalar engine (bf16 out, per-partition scalar multiply)
        nc.scalar.mul(block, xp, masks[0][:])

    def shuffle_step(block, xp, masks, r):
        # accumulate rotation r: out[j] += masks[r] * x[(j-r)%6], which for
        # fixed r is 2 contiguous pieces
        cut = (NROT - r) * HW
        nc.vector.scalar_tensor_tensor(
            block[:, r * HW:NROT * HW], xp[:, 0:cut], masks[r][:],
            block[:, r * HW:NROT * HW], op0=ALU.mult, op1=ALU.add)
        nc.vector.scalar_tensor_tensor(
            block[:, 0:r * HW], xp[:, cut:NROT * HW], masks[r][:],
            block[:, 0:r * HW], op0=ALU.mult, op1=ALU.add)

    def emit_chsum(slot, xp):
        # sum of 6 non-negative channels as sequential GpSimd f32 adds;
        # only the final DVE copy rounds to bf16.
        nc.gpsimd.tensor_add(csum[:], xp[:, 0:HW], xp[:, HW:2 * HW])
        for k in range(2, NROT):
            nc.gpsimd.tensor_add(csum[:], csum[:], xp[:, k * HW:(k + 1) * HW])
        nc.vector.tensor_copy(slot, csum[:])

    # ---- emission units: (cost, fn) where cost ~ DVE us impact ----
    stage_tiles = {}
    pending_writes = []  # deferred (out_cols, tile) DMA stores

    def stage(ck):
        if ck not in stage_tiles:
            n_ch = STAGE_CHUNKS[ck][1]
            stage_tiles[ck] = stage_pool.tile(
                [BS, n_ch * HW], BF16, tag="stage", name=f"stage{ck}")
        return stage_tiles[ck]

    def close(ck):
        start_ch, n_ch = STAGE_CHUNKS[ck]
        pending_writes.append(
            (slice(start_ch * HW, (start_ch + n_ch) * HW), stage_tiles[ck]))

    units = []

    def u_singles01():  # ch128 obstacle*obs, ch129 ocur*obs (merged)
        nc.vector.tensor_mul(stage(0)[:, 0:2 * HW], sing_f[:, 0:2 * HW],
                             obs_rep[:, 0:2 * HW])

    def u_obsobs():  # ch130
        nc.vector.tensor_mul(stage(0)[:, 2 * HW:3 * HW], obs_t, obs_t)
        close(0)

    units.append((1, u_singles01))
    units.append((1, u_obsobs))

    def shuffle_units(ck, xp, masks):
        blk = [None]

        def init():
            blk[0] = stage(ck)
            shuffle_init(blk[0][:], xp, masks)

        units.append((0, init))  # scalar-engine work, ~free for DVE
        for r in range(1, NROT):
            def step(r=r):
                shuffle_step(blk[0], xp, masks, r)
            units.append((2, step))
            if r == NROT - 1:
                units.append((0, lambda ck=ck: close(ck)))

    shuffle_units(1, vis_t, Rh)  # ch131..136

    def u_chsum_vis():  # ch137
        emit_chsum(stage(2)[:, 0:HW], vis_t)

    def u_singles23():  # ch138 leader*obs, ch139 follower*obs (merged)
        nc.vector.tensor_mul(stage(2)[:, HW:3 * HW], sing_f[:, 2 * HW:4 * HW],
                             obs_rep[:, 0:2 * HW])
        close(2)

    units.append((1, u_chsum_vis))
    units.append((1, u_singles23))

    shuffle_units(3, atgt_t, Rh)  # ch140..145
    shuffle_units(4, ptgt_t, R)   # ch146..151

    def u_chsum_atgt():  # ch152 (*0.5)
        slot = stage(5)[:, 0:HW]
        emit_chsum(slot, atgt_t)
        nc.vector.tensor_scalar_mul(slot, slot, 0.5)

    def u_chsum_ptgt():  # ch153
        emit_chsum(stage(5)[:, HW:2 * HW], ptgt_t)

    def u_ones():  # ch154
        nc.vector.memset(stage(5)[:, 2 * HW:3 * HW], 1.0)
        close(5)

    units.append((1, u_chsum_atgt))
    units.append((1, u_chsum_ptgt))
    units.append((1, u_ones))

    def u_compass():  # ch155..160 on the Scalar engine
        for r in range(NROT):
            nc.scalar.activation(
                stage(6)[:, r * HW:(r + 1) * HW], obs_t,
                mybir.ActivationFunctionType.Identity,
                bias=R[r][:], scale=0.0)
        close(6)

    units.append((0, u_compass))

    def emit_small(budget):
        while budget > 0 and units:
            cost, fn = units.pop(0)
            fn()
            budget -= cost

    # ---- env stream: one interleaved [s_chunk | d_chunk] load per chunk ----
    # Loads go on the SP HWDGE ring, stores on the Activation ring, deferred
    # one iteration.
    w = ENV_CHUNK * HW
    env_total = EMB // ENV_CHUNK
    for c in range(env_total):
        sd_tile = env_pool.tile([BS, 2 * w], F32, tag="env")
        nc.sync.dma_start(sd_tile[:], t_in["env_il"][:, c * 2 * w:(c + 1) * 2 * w])
        for out_cols, tile_ in pending_writes:
            nc.scalar.dma_start(t_out[:, out_cols], tile_[:])
        pending_writes.clear()
        nc.vector.tensor_add(sd_tile[:, 0:w], sd_tile[:, 0:w],
                             sd_tile[:, w:2 * w])
        o_tile = env_o_pool.tile([BS, w], BF16, tag="env_o")
        nc.vector.tensor_mul(o_tile[:], sd_tile[:, 0:w], obs_rep[:])
        pending_writes.append((slice(c * w, (c + 1) * w), o_tile))
        if c >= 2:
            emit_small(2)
    emit_small(sum(u[0] for u in units) + 1)
    for out_cols, tile_ in pending_writes:
        nc.scalar.dma_start(t_out[:, out_cols], tile_[:])
    pending_writes.clear()


def build_nc():
    nc = bacc.Bacc("TRN2", target_bir_lowering=False, debug=False)
    t_in = {
        "env_il": nc.dram_tensor(
            "env_il", [BS, 2 * EMB * HW], F32, kind="ExternalInput"),
        "small_pack": nc.dram_tensor(
            "small_pack", [BS, PACK_W], BF16, kind="ExternalInput"),
    }
    t_out = nc.dram_tensor("out", [BS, NCH * HW], BF16, kind="ExternalOutput")
    with tile.TileContext(nc) as tc, ExitStack() as ctx:
        build_body(nc, tc, ctx, t_in, t_out)
    nc.compile()
    return nc


def make_in_maps(inputs):
    arrs = {k: np.asarray(v) for k, v in inputs.items()}
    w = ENV_CHUNK * HW
    n_chunk = EMB // ENV_CHUNK
    s = arrs["embedded_static"].reshape(B, n_chunk, 1, w)
    d = arrs["embedded_dynamic"].reshape(B, n_chunk, 1, w)
    env_il = np.ascontiguousarray(
        np.concatenate([s, d], axis=2)).reshape(B, 2 * EMB * HW)
    src = {
        "obs": arrs["observability_in_memory"].reshape(B, HW),
        "obstacle": arrs["obstacle_mask"].reshape(B, HW),
        "ocur": arrs["observability_current"].reshape(B, HW),
        "leader": arrs["leader_location"].reshape(B, HW),
        "follower": arrs["follower_location"].reshape(B, HW),
        "vis": arrs["previous_visitations"].reshape(B, NROT * HW),
        "atgt": arrs["all_previous_targets"].reshape(B, NROT * HW),
        "ptgt": arrs["previous_target"].reshape(B, NROT * HW),
        # rotations 0..5 are exactly representable as bf16 float values
        "rot": arrs["rotations"].reshape(B, 1).astype(np.float32),
    }
    flat = {
        "env_il": env_il,
        "small_pack": np.concatenate(
            [src[name] for name, _ in PACK_LAYOUT], axis=1).astype(NPBF),
    }
    return [
        {k: v[i * BS:(i + 1) * BS] for k, v in flat.items()}
        for i in range(N_CORES)
    ]


def kernel(**inputs) -> np.ndarray:
    nc = build_nc()
    in_maps = make_in_maps(inputs)
    res = run_bass_kernel_spmd(nc, in_maps, list(range(N_CORES)))
    return np.concatenate(
        [r["out"].astype(np.float32).reshape(BS, NCH, 25, 25)
         for r in res.results], axis=0)


if __name__ == "__main__":
    rng = np.random.default_rng(0)
    demo = {
        "embedded_static": rng.standard_normal((B, EMB, 25, 25), np.float32),
        "embedded_dynamic": rng.standard_normal((B, EMB, 25, 25), np.float32),
        "obstacle_mask": rng.random((B, 25, 25), dtype=np.float32),
        "observability_current": rng.random((B, 25, 25), dtype=np.float32),
        "observability_in_memory": rng.random((B, 25, 25), dtype=np.float32),
        "previous_visitations": rng.random((B, NROT, 25, 25), dtype=np.float32),
        "all_previous_targets": rng.random((B, NROT, 25, 25), dtype=np.float32),
        "previous_target": rng.random((B, NROT, 25, 25), dtype=np.float32),
        "leader_location": rng.random((B, 25, 25), dtype=np.float32),
        "follower_location": rng.random((B, 25, 25), dtype=np.float32),
        "rotations": rng.integers(0, NROT, (B,), dtype=np.int32),
    }
    out = kernel(**demo)
    print("out", out.shape, out.dtype)
